# revision 16
# baseline (speedup 1.0000x reference)
"""Causal self-attention (B=2, T=2048, C=1024, H=16) on 8 trn2 NeuronCores.

Sharding: core i handles batch b = i // 4 and head-group hg = i % 4
(4 heads each). Data-parallel over B, tensor-parallel over heads:
each core computes q/k/v for its 4 heads, full causal attention locally,
and a partial projection out = y_heads @ W_proj[rows]; the host sums the
4 partials per batch. No collectives.

Layout trick: everything is computed in "transposed space" so no on-chip
transposes are needed:
  - host passes xT = x[b].T  [C, T]
  - qT/kT [d, T] come straight out of the qkv matmul (W as lhsT, xT as rhs)
  - v [T, d] natural (xT as lhsT, W_v as rhs), augmented with a ones column
  - scoresT[k, q] = kT_slice.T @ qT_slice  (contraction over d=64)
  - pT = exp(scale * (scoresT + mask))     (ACT, PSUM -> SBUF)
  - yT[d, q] (+denom row) accumulates v_aug.T @ pT over k-tiles
  - y = yT[0:64] * (1 / denom row), broadcast via gpsimd partition_broadcast
  - out[t, c] partial = yT.T @ W_proj_rows  (yT as lhsT, natural W_proj rhs)

All matmuls run as float32r (fp32 data, full-rate PE mode, ~1e-4 rel err).
"""

import sys

import numpy as np

sys.path.insert(0, "/opt/trn_rl_repo")

B, T, C = 2, 2048, 1024
N_HEAD = 16
D = C // N_HEAD          # 64
HPC = N_HEAD // 4        # 4 heads per core
CS = HPC * D             # 256 = per-core slice width of q/k/v
NCHUNK = C // 128        # 8 contraction chunks over C
NT = T // 128            # 16 row tiles
NQ = T // 512            # 4 query tiles of 512
SCALE = 1.0 / np.sqrt(D)
NEG = -1.0e30

_CACHE = {}


def _build():
    import concourse.bacc as bacc
    import concourse.mybir as mybir
    import concourse.tile as tile

    F32 = mybir.dt.float32
    F32R = mybir.dt.float32r

    nc = bacc.Bacc("TRN2", target_bir_lowering=False, debug=False, num_devices=8)

    xT = nc.dram_tensor("xT", [C, T], F32R, kind="ExternalInput").ap()
    wq = nc.dram_tensor("wq", [C, CS], F32R, kind="ExternalInput").ap()
    wk = nc.dram_tensor("wk", [C, CS], F32R, kind="ExternalInput").ap()
    wv = nc.dram_tensor("wv", [C, CS], F32R, kind="ExternalInput").ap()
    wp = nc.dram_tensor("wp", [CS, C], F32R, kind="ExternalInput").ap()
    mask = nc.dram_tensor("mask", [128, T], F32, kind="ExternalInput").ap()
    out = nc.dram_tensor("out", [T, C], F32, kind="ExternalOutput").ap()

    with tile.TileContext(nc) as tc:
        with (
            tc.tile_pool(name="persist", bufs=1) as pp,
            tc.tile_pool(name="consts", bufs=1) as cp,
        ):
            # persistent SBUF tensors
            qt = [pp.tile([128, T], F32R, name=f"qt{m}", tag=f"qt{m}") for m in range(2)]
            # Per-head kT, zero-padded to the full 128-partition contraction:
            # head h occupies rows (h%2)*64..+64, the other 64 rows are 0.
            # This keeps scores matmuls at K=128 (K=64 with changing weights
            # serializes the weight load: ~435ns/MM vs ~234ns at K=128).
            ktp = [pp.tile([128, T], F32R, name=f"ktp{h}", tag=f"ktp{h}")
                   for h in range(HPC)]
            yt = [pp.tile([128, T], F32R, name=f"yt{m}", tag=f"yt{m}") for m in range(2)]
            # v_aug per row-tile: [128, 4 heads, 65] (col 64 = ones)
            va = [pp.tile([128, HPC, D + 1], F32R, name=f"va{t}", tag=f"va{t}") for t in range(NT)]
            mk = cp.tile([128, T], F32, tag="mask")
            wpt = cp.tile([128, 2, C], F32R, tag="wp")
            ones_f32 = cp.tile([128, HPC], F32, tag="ones")
            nc.gpsimd.memset(ones_f32[:], 1.0)
            # Dummy exp: pulls the ~2.7us ACT table load off the critical path
            warm = cp.tile([128, 1], F32, tag="warm")
            nc.scalar.activation(warm[:], ones_f32[:, 0:1],
                                 mybir.ActivationFunctionType.Exp, scale=1.0)
            zeros_f32 = cp.tile([64, T], F32, tag="zeros")
            nc.gpsimd.memset(zeros_f32[:], 0.0)
            for h in range(HPC):
                off = 64 - (h % 2) * 64  # the half that stays zero
                nc.vector.tensor_copy(ktp[h][off:off + 64, :], zeros_f32[:])

            nc.sync.dma_start(mk[:], mask[:])
            nc.sync.dma_start(
                wpt[:],
                wp.rearrange("(c p) n -> p c n", p=128),
            )

            # ---------------- phase B: qkv ----------------
            with (
                tc.tile_pool(name="xw", bufs=1) as xw,
                tc.tile_pool(name="psqk", bufs=4, space="PSUM") as psqk,
                tc.tile_pool(name="psv", bufs=4, space="PSUM") as psv,
            ):
                xt = xw.tile([128, NCHUNK, T], F32R, tag="xt")
                wqt = xw.tile([128, NCHUNK, CS], F32R, tag="wq")
                wkt = xw.tile([128, NCHUNK, CS], F32R, tag="wk")
                wvt = xw.tile([128, NCHUNK, CS], F32R, tag="wv")
                for c in range(NCHUNK):
                    nc.sync.dma_start(xt[:, c], xT[c * 128:(c + 1) * 128, :])
                nc.sync.dma_start(wqt[:], wq.rearrange("(c p) n -> p c n", p=128))
                nc.sync.dma_start(wkt[:], wk.rearrange("(c p) n -> p c n", p=128))
                nc.sync.dma_start(wvt[:], wv.rearrange("(c p) n -> p c n", p=128))

                # qT / kT: out [128 (2 heads), T] per m-tile
                for m in range(2):
                    for w_all, which in ((wqt, "q"), (wkt, "k")):
                        for ns in range(T // 512):
                            ps = psqk.tile([128, 512], F32, tag="psqk")
                            for c in range(NCHUNK):
                                nc.tensor.matmul(
                                    ps[:],
                                    w_all[:, c, m * 128:(m + 1) * 128],
                                    xt[:, c, ns * 512:(ns + 1) * 512],
                                    start=(c == 0),
                                    stop=(c == NCHUNK - 1),
                                )
                            sl = slice(ns * 512, (ns + 1) * 512)
                            if which == "q":
                                # ACT is idle during phase B; offload q evicts
                                nc.scalar.copy(qt[m][:, sl], ps[:])
                            else:
                                nc.vector.tensor_copy(
                                    ktp[2 * m][0:64, sl], ps[0:64, :]
                                )
                                nc.vector.tensor_copy(
                                    ktp[2 * m + 1][64:128, sl], ps[64:128, :]
                                )
                # v natural [T, 256] -> v_aug tiles
                for t in range(NT):
                    ps = psv.tile([128, CS], F32, tag="psv")
                    for c in range(NCHUNK):
                        nc.tensor.matmul(
                            ps[:],
                            xt[:, c, t * 128:(t + 1) * 128],
                            wvt[:, c, :],
                            start=(c == 0),
                            stop=(c == NCHUNK - 1),
                        )
                    nc.vector.tensor_copy(
                        va[t][:, :, 0:D],
                        ps[:].rearrange("p (h d) -> p h d", h=HPC),
                    )
                    nc.vector.tensor_copy(va[t][:, :, D], ones_f32[:])

            # ---------------- phase C: attention ----------------
            # Two heads run in interleaved "lanes" so the PE never stalls on
            # ACT's exp: while lane A's exp runs, the PE does lane B's
            # scores, and vice versa. Keeps PE dense -> HAM stays warm.
            with (
                tc.tile_pool(name="pt", bufs=6) as ptp,
                tc.tile_pool(name="sm", bufs=4) as smp,
                tc.tile_pool(name="pss", bufs=3, space="PSUM") as pss_p,
                tc.tile_pool(name="psy", bufs=2, space="PSUM") as psy_p,
            ):
                def scores_exp(h, j, g):
                    hq = h // 2
                    pss = pss_p.tile([128, 1024], F32, tag="pss", name="pss")
                    for i in range(2):
                        kb = g * 2 + i
                        nc.tensor.matmul(
                            pss[:, i * 512:(i + 1) * 512],
                            ktp[h][:, kb * 128:(kb + 1) * 128],
                            qt[hq][:, j * 512:(j + 1) * 512],
                            start=True,
                            stop=True,
                        )
                    for i in range(2):
                        kb = g * 2 + i
                        di = kb - 4 * j  # index within diagonal 512 region
                        if di >= 0:
                            w = 128 * (di + 1)
                            nc.vector.tensor_add(
                                pss[:, i * 512:i * 512 + w],
                                pss[:, i * 512:i * 512 + w],
                                mk[:, di * 512:di * 512 + w],
                            )
                    pt = ptp.tile([128, 1024], F32R, tag="pt", name="pt")
                    nc.scalar.activation(
                        pt[:], pss[:],
                        mybir.ActivationFunctionType.Exp,
                        scale=float(SCALE),
                    )
                    return pt

                def y_acc(h, j, g, psy, pt):
                    nkb = 4 * (j + 1)
                    for i in range(2):
                        kb = g * 2 + i
                        nc.tensor.matmul(
                            psy[:],
                            va[kb][:, h, :],
                            pt[:, i * 512:(i + 1) * 512],
                            start=(kb == 0),
                            stop=(kb == nkb - 1),
                        )

                def divide(h, j, psy):
                    # Copy y and the denominator out fast (releases the psy
                    # bank), then run the slow recip/broadcast chain off the
                    # critical path.
                    hq, ho = h // 2, (h % 2) * 64
                    den = smp.tile([1, 512], F32, tag="den", name="den")
                    nc.vector.tensor_copy(den[:], psy[D:D + 1, :])
                    yu = smp.tile([D, 512], F32, tag="yu", name="yu")
                    nc.vector.tensor_copy(yu[:], psy[0:D, :])
                    rec = smp.tile([1, 512], F32, tag="rec", name="rec")
                    nc.vector.reciprocal_approx_fast(rec[:], den[:])
                    bc = smp.tile([D, 512], F32, tag="bc", name="bc")
                    nc.gpsimd.partition_broadcast(bc[:], rec[:])
                    nc.vector.tensor_mul(
                        yt[hq][ho:ho + 64, j * 512:(j + 1) * 512],
                        yu[:],
                        bc[:],
                    )

                def proj(j, pop):
                    # project the j-block of rows (t-tiles 4j..4j+3) once all
                    # 4 heads' yt columns for this j are divided
                    for t in range(4 * j, 4 * j + 4):
                        for nb in range(2):
                            ps = pss_p.tile([128, 512], F32, tag="pss",
                                            name="pso")
                            for cc in range(2):
                                nc.tensor.matmul(
                                    ps[:],
                                    yt[cc][:, t * 128:(t + 1) * 128],
                                    wpt[:, cc, nb * 512:(nb + 1) * 512],
                                    start=(cc == 0),
                                    stop=(cc == 1),
                                )
                            ot = pop.tile([128, 512], F32, tag="po", name="po")
                            if (t + nb) % 2 == 0:
                                nc.vector.tensor_copy(ot[:], ps[:])
                            else:
                                nc.scalar.copy(ot[:], ps[:])
                            nc.sync.dma_start(
                                out[t * 128:(t + 1) * 128,
                                    nb * 512:(nb + 1) * 512],
                                ot[:],
                            )

                with tc.tile_pool(name="po", bufs=4) as pop:
                    pending_proj = None
                    for j in range(NQ):
                        nG = 2 * (j + 1)  # groups of 2 causal k-tiles
                        for hp in range(2):
                            hA, hB = 2 * hp, 2 * hp + 1
                            psyA = psy_p.tile([D + 1, 512], F32, tag="psy",
                                              name="psyA")
                            psyB = psy_p.tile([D + 1, 512], F32, tag="psy",
                                              name="psyB")
                            ptB_prev = None
                            for g in range(nG):
                                ptA = scores_exp(hA, j, g)
                                if ptB_prev is not None:
                                    y_acc(hB, j, g - 1, psyB, ptB_prev)
                                if g == 1 and hp == 0 and pending_proj is not None:
                                    proj(pending_proj, pop)
                                    pending_proj = None
                                ptB = scores_exp(hB, j, g)
                                y_acc(hA, j, g, psyA, ptA)
                                ptB_prev = ptB
                            y_acc(hB, j, nG - 1, psyB, ptB_prev)
                            divide(hA, j, psyA)
                            divide(hB, j, psyB)
                        pending_proj = j
                    proj(pending_proj, pop)

    nc.compile()
    return nc


def _causal_mask():
    m = np.zeros((128, T), dtype=np.float32)
    kk = np.arange(128)[:, None]
    for i in range(4):
        qq = np.arange(512)[None, :]
        blk = np.where(i * 128 + kk <= qq, 0.0, NEG).astype(np.float32)
        m[:, i * 512:(i + 1) * 512] = blk
    return m


def _get_nc():
    if "nc" not in _CACHE:
        _CACHE["nc"] = _build()
    return _CACHE["nc"]


def _run(x, W_qkv, W_proj, trace=False, trace_cores=None):
    from concourse.bass_utils import run_bass_kernel_spmd

    x = np.asarray(x, dtype=np.float32)
    W_qkv = np.asarray(W_qkv, dtype=np.float32)
    W_proj = np.asarray(W_proj, dtype=np.float32)

    nc = _get_nc()
    mask = _causal_mask()
    in_maps = []
    for core in range(8):
        b, hg = core // 4, core % 4
        sl = slice(hg * CS, (hg + 1) * CS)
        in_maps.append({
            "xT": np.ascontiguousarray(x[b].T),
            "wq": np.ascontiguousarray(W_qkv[:, sl]),
            "wk": np.ascontiguousarray(W_qkv[:, C + hg * CS:C + (hg + 1) * CS]),
            "wv": np.ascontiguousarray(W_qkv[:, 2 * C + hg * CS:2 * C + (hg + 1) * CS]),
            "wp": np.ascontiguousarray(W_proj[sl, :]),
            "mask": mask,
        })

    res = run_bass_kernel_spmd(
        nc, in_maps, list(range(8)), trace=trace, trace_cores=trace_cores
    )
    outp = np.zeros((B, T, C), dtype=np.float32)
    for core in range(8):
        outp[core // 4] += res.results[core]["out"]
    return outp, res


def kernel(x, W_qkv, W_proj):
    outp, _ = _run(x, W_qkv, W_proj)
    return outp


# revision 17
# speedup vs baseline: 1.0600x; 1.0600x over previous
"""Causal self-attention (B=2, T=2048, C=1024, H=16) on 8 trn2 NeuronCores.

Sharding: core i handles batch b = i // 4 and head-group hg = i % 4
(4 heads each). Data-parallel over B, tensor-parallel over heads:
each core computes q/k/v for its 4 heads, full causal attention locally,
and a partial projection out = y_heads @ W_proj[rows]; the host sums the
4 partials per batch. No collectives.

Layout trick: everything is computed in "transposed space" so no on-chip
transposes are needed:
  - host passes xT = x[b].T  [C, T]
  - qT/kT [d, T] come straight out of the qkv matmul (W as lhsT, xT as rhs)
  - v [T, d] natural (xT as lhsT, W_v as rhs), augmented with a ones column
  - scoresT[k, q] = kT_slice.T @ qT_slice  (contraction over d=64)
  - pT = exp(scale * (scoresT + mask))     (ACT, PSUM -> SBUF)
  - yT[d, q] (+denom row) accumulates v_aug.T @ pT over k-tiles
  - y = yT[0:64] * (1 / denom row), broadcast via gpsimd partition_broadcast
  - out[t, c] partial = yT.T @ W_proj_rows  (yT as lhsT, natural W_proj rhs)

All matmuls run as float32r (fp32 data, full-rate PE mode, ~1e-4 rel err).
"""

import sys

import numpy as np

sys.path.insert(0, "/opt/trn_rl_repo")

B, T, C = 2, 2048, 1024
N_HEAD = 16
D = C // N_HEAD          # 64
HPC = N_HEAD // 4        # 4 heads per core
CS = HPC * D             # 256 = per-core slice width of q/k/v
NCHUNK = C // 128        # 8 contraction chunks over C
NT = T // 128            # 16 row tiles
NQ = T // 512            # 4 query tiles of 512
SCALE = 1.0 / np.sqrt(D)
NEG = -1.0e30

_CACHE = {}


def _build():
    import concourse.bacc as bacc
    import concourse.mybir as mybir
    import concourse.tile as tile

    F32 = mybir.dt.float32
    F32R = mybir.dt.float32r

    nc = bacc.Bacc("TRN2", target_bir_lowering=False, debug=False, num_devices=8)

    xT = nc.dram_tensor("xT", [C, T], F32R, kind="ExternalInput").ap()
    wq = nc.dram_tensor("wq", [C, CS], F32R, kind="ExternalInput").ap()
    wk = nc.dram_tensor("wk", [C, CS], F32R, kind="ExternalInput").ap()
    wv = nc.dram_tensor("wv", [C, CS], F32R, kind="ExternalInput").ap()
    wp = nc.dram_tensor("wp", [CS, C], F32R, kind="ExternalInput").ap()
    mask = nc.dram_tensor("mask", [128, T], F32, kind="ExternalInput").ap()
    out = nc.dram_tensor("out", [T, C], F32, kind="ExternalOutput").ap()

    with tile.TileContext(nc) as tc:
        with (
            tc.tile_pool(name="persist", bufs=1) as pp,
            tc.tile_pool(name="consts", bufs=1) as cp,
        ):
            # persistent SBUF tensors
            qt = [pp.tile([128, T], F32R, name=f"qt{m}", tag=f"qt{m}") for m in range(2)]
            # Per-head kT, zero-padded to the full 128-partition contraction:
            # head h occupies rows (h%2)*64..+64, the other 64 rows are 0.
            # This keeps scores matmuls at K=128 (K=64 with changing weights
            # serializes the weight load: ~435ns/MM vs ~234ns at K=128).
            ktp = [pp.tile([128, T], F32R, name=f"ktp{h}", tag=f"ktp{h}")
                   for h in range(HPC)]
            yt = [pp.tile([128, T], F32R, name=f"yt{m}", tag=f"yt{m}") for m in range(2)]
            # v_aug per row-tile: [128, 4 heads, 65] (col 64 = ones)
            va = [pp.tile([128, HPC, D + 1], F32R, name=f"va{t}", tag=f"va{t}") for t in range(NT)]
            mk = cp.tile([128, T], F32, tag="mask")
            wpt = cp.tile([128, 2, C], F32R, tag="wp")
            ones_f32 = cp.tile([128, HPC], F32, tag="ones")
            nc.gpsimd.memset(ones_f32[:], 1.0)
            # Dummy exp: pulls the ~2.7us ACT table load off the critical path
            warm = cp.tile([128, 1], F32, tag="warm")
            nc.scalar.activation(warm[:], ones_f32[:, 0:1],
                                 mybir.ActivationFunctionType.Exp, scale=1.0)
            zeros_f32 = cp.tile([64, T], F32, tag="zeros")
            nc.gpsimd.memset(zeros_f32[:], 0.0)
            for h in range(HPC):
                off = 64 - (h % 2) * 64  # the half that stays zero
                nc.vector.tensor_copy(ktp[h][off:off + 64, :], zeros_f32[:])

            nc.sync.dma_start(mk[:], mask[:])
            nc.sync.dma_start(
                wpt[:],
                wp.rearrange("(c p) n -> p c n", p=128),
            )

            # ---------------- phase B: qkv ----------------
            # x streams in 512-column slices so the PE starts after ~2MB of
            # DMA instead of the full 8MB; qk/v compute per slice overlaps
            # the next slice's DMA.
            with (
                tc.tile_pool(name="xw", bufs=1) as xw,
                tc.tile_pool(name="psqk", bufs=4, space="PSUM") as psqk,
                tc.tile_pool(name="psv", bufs=4, space="PSUM") as psv,
            ):
                wqt = xw.tile([128, NCHUNK, CS], F32R, tag="wq")
                wkt = xw.tile([128, NCHUNK, CS], F32R, tag="wk")
                wvt = xw.tile([128, NCHUNK, CS], F32R, tag="wv")
                nc.sync.dma_start(wqt[:], wq.rearrange("(c p) n -> p c n", p=128))
                nc.sync.dma_start(wkt[:], wk.rearrange("(c p) n -> p c n", p=128))
                nc.sync.dma_start(wvt[:], wv.rearrange("(c p) n -> p c n", p=128))
                xts = []
                for ns in range(T // 512):
                    xtn = xw.tile([128, NCHUNK, 512], F32R,
                                  name=f"xt{ns}", tag=f"xt{ns}")
                    xts.append(xtn)
                    for c in range(NCHUNK):
                        nc.sync.dma_start(
                            xtn[:, c],
                            xT[c * 128:(c + 1) * 128, ns * 512:(ns + 1) * 512],
                        )

                for ns in range(T // 512):
                    xtn = xts[ns]
                    sl = slice(ns * 512, (ns + 1) * 512)
                    for m in range(2):
                        for w_all, which in ((wqt, "q"), (wkt, "k")):
                            ps = psqk.tile([128, 512], F32, tag="psqk",
                                           name="psqk")
                            for c in range(NCHUNK):
                                nc.tensor.matmul(
                                    ps[:],
                                    w_all[:, c, m * 128:(m + 1) * 128],
                                    xtn[:, c, :],
                                    start=(c == 0),
                                    stop=(c == NCHUNK - 1),
                                )
                            if which == "q":
                                # ACT is idle during phase B
                                nc.scalar.copy(qt[m][:, sl], ps[:])
                            else:
                                nc.vector.tensor_copy(
                                    ktp[2 * m][0:64, sl], ps[0:64, :]
                                )
                                nc.vector.tensor_copy(
                                    ktp[2 * m + 1][64:128, sl], ps[64:128, :]
                                )
                    # v for the 4 row-tiles inside this slice
                    for t in range(4 * ns, 4 * ns + 4):
                        ps = psv.tile([128, CS], F32, tag="psv", name="psv")
                        for c in range(NCHUNK):
                            nc.tensor.matmul(
                                ps[:],
                                xtn[:, c, (t % 4) * 128:(t % 4 + 1) * 128],
                                wvt[:, c, :],
                                start=(c == 0),
                                stop=(c == NCHUNK - 1),
                            )
                        nc.vector.tensor_copy(
                            va[t][:, :, 0:D],
                            ps[:].rearrange("p (h d) -> p h d", h=HPC),
                        )
                        nc.vector.tensor_copy(va[t][:, :, D], ones_f32[:])

            # ---------------- phase C: attention ----------------
            # Two heads run in interleaved "lanes" so the PE never stalls on
            # ACT's exp: while lane A's exp runs, the PE does lane B's
            # scores, and vice versa. Keeps PE dense -> HAM stays warm.
            with (
                tc.tile_pool(name="pt", bufs=6) as ptp,
                tc.tile_pool(name="sm", bufs=4) as smp,
                tc.tile_pool(name="pss", bufs=3, space="PSUM") as pss_p,
                tc.tile_pool(name="psy", bufs=2, space="PSUM") as psy_p,
            ):
                def scores_exp(h, j, g):
                    hq = h // 2
                    pss = pss_p.tile([128, 1024], F32, tag="pss", name="pss")
                    for i in range(2):
                        kb = g * 2 + i
                        nc.tensor.matmul(
                            pss[:, i * 512:(i + 1) * 512],
                            ktp[h][:, kb * 128:(kb + 1) * 128],
                            qt[hq][:, j * 512:(j + 1) * 512],
                            start=True,
                            stop=True,
                        )
                    for i in range(2):
                        kb = g * 2 + i
                        di = kb - 4 * j  # index within diagonal 512 region
                        if di >= 0:
                            w = 128 * (di + 1)
                            nc.vector.tensor_add(
                                pss[:, i * 512:i * 512 + w],
                                pss[:, i * 512:i * 512 + w],
                                mk[:, di * 512:di * 512 + w],
                            )
                    pt = ptp.tile([128, 1024], F32R, tag="pt", name="pt")
                    nc.scalar.activation(
                        pt[:], pss[:],
                        mybir.ActivationFunctionType.Exp,
                        scale=float(SCALE),
                    )
                    return pt

                def y_acc(h, j, g, psy, pt):
                    nkb = 4 * (j + 1)
                    for i in range(2):
                        kb = g * 2 + i
                        nc.tensor.matmul(
                            psy[:],
                            va[kb][:, h, :],
                            pt[:, i * 512:(i + 1) * 512],
                            start=(kb == 0),
                            stop=(kb == nkb - 1),
                        )

                def divide(h, j, psy):
                    # Copy y and the denominator out fast (releases the psy
                    # bank), then run the slow recip/broadcast chain off the
                    # critical path.
                    hq, ho = h // 2, (h % 2) * 64
                    den = smp.tile([1, 512], F32, tag="den", name="den")
                    nc.vector.tensor_copy(den[:], psy[D:D + 1, :])
                    yu = smp.tile([D, 512], F32, tag="yu", name="yu")
                    nc.vector.tensor_copy(yu[:], psy[0:D, :])
                    rec = smp.tile([1, 512], F32, tag="rec", name="rec")
                    nc.vector.reciprocal_approx_fast(rec[:], den[:])
                    bc = smp.tile([D, 512], F32, tag="bc", name="bc")
                    nc.gpsimd.partition_broadcast(bc[:], rec[:])
                    nc.vector.tensor_mul(
                        yt[hq][ho:ho + 64, j * 512:(j + 1) * 512],
                        yu[:],
                        bc[:],
                    )

                def proj(j, pop):
                    # project the j-block of rows (t-tiles 4j..4j+3) once all
                    # 4 heads' yt columns for this j are divided
                    for t in range(4 * j, 4 * j + 4):
                        for nb in range(2):
                            ps = pss_p.tile([128, 512], F32, tag="pss",
                                            name="pso")
                            for cc in range(2):
                                nc.tensor.matmul(
                                    ps[:],
                                    yt[cc][:, t * 128:(t + 1) * 128],
                                    wpt[:, cc, nb * 512:(nb + 1) * 512],
                                    start=(cc == 0),
                                    stop=(cc == 1),
                                )
                            ot = pop.tile([128, 512], F32, tag="po", name="po")
                            if (t + nb) % 2 == 0:
                                nc.vector.tensor_copy(ot[:], ps[:])
                            else:
                                nc.scalar.copy(ot[:], ps[:])
                            nc.sync.dma_start(
                                out[t * 128:(t + 1) * 128,
                                    nb * 512:(nb + 1) * 512],
                                ot[:],
                            )

                with tc.tile_pool(name="po", bufs=4) as pop:
                    pending_proj = None
                    for j in range(NQ):
                        nG = 2 * (j + 1)  # groups of 2 causal k-tiles
                        for hp in range(2):
                            hA, hB = 2 * hp, 2 * hp + 1
                            psyA = psy_p.tile([D + 1, 512], F32, tag="psy",
                                              name="psyA")
                            psyB = psy_p.tile([D + 1, 512], F32, tag="psy",
                                              name="psyB")
                            ptB_prev = None
                            for g in range(nG):
                                ptA = scores_exp(hA, j, g)
                                if ptB_prev is not None:
                                    y_acc(hB, j, g - 1, psyB, ptB_prev)
                                if g == 1 and hp == 0 and pending_proj is not None:
                                    proj(pending_proj, pop)
                                    pending_proj = None
                                ptB = scores_exp(hB, j, g)
                                y_acc(hA, j, g, psyA, ptA)
                                ptB_prev = ptB
                            y_acc(hB, j, nG - 1, psyB, ptB_prev)
                            divide(hA, j, psyA)
                            divide(hB, j, psyB)
                        pending_proj = j
                    proj(pending_proj, pop)

    nc.compile()
    return nc


def _causal_mask():
    m = np.zeros((128, T), dtype=np.float32)
    kk = np.arange(128)[:, None]
    for i in range(4):
        qq = np.arange(512)[None, :]
        blk = np.where(i * 128 + kk <= qq, 0.0, NEG).astype(np.float32)
        m[:, i * 512:(i + 1) * 512] = blk
    return m


def _get_nc():
    if "nc" not in _CACHE:
        _CACHE["nc"] = _build()
    return _CACHE["nc"]


def _run(x, W_qkv, W_proj, trace=False, trace_cores=None):
    from concourse.bass_utils import run_bass_kernel_spmd

    x = np.asarray(x, dtype=np.float32)
    W_qkv = np.asarray(W_qkv, dtype=np.float32)
    W_proj = np.asarray(W_proj, dtype=np.float32)

    nc = _get_nc()
    mask = _causal_mask()
    in_maps = []
    for core in range(8):
        b, hg = core // 4, core % 4
        sl = slice(hg * CS, (hg + 1) * CS)
        in_maps.append({
            "xT": np.ascontiguousarray(x[b].T),
            "wq": np.ascontiguousarray(W_qkv[:, sl]),
            "wk": np.ascontiguousarray(W_qkv[:, C + hg * CS:C + (hg + 1) * CS]),
            "wv": np.ascontiguousarray(W_qkv[:, 2 * C + hg * CS:2 * C + (hg + 1) * CS]),
            "wp": np.ascontiguousarray(W_proj[sl, :]),
            "mask": mask,
        })

    res = run_bass_kernel_spmd(
        nc, in_maps, list(range(8)), trace=trace, trace_cores=trace_cores
    )
    outp = np.zeros((B, T, C), dtype=np.float32)
    for core in range(8):
        outp[core // 4] += res.results[core]["out"]
    return outp, res


def kernel(x, W_qkv, W_proj):
    outp, _ = _run(x, W_qkv, W_proj)
    return outp
